# revision 60
# baseline (speedup 1.0000x reference)
"""CrossViT fused block on 8 TRN2 NeuronCores.

Sharding: 2 branches (vis-output / ir-output) x 4-way token split -> 8 cores,
no collectives. Each core computes 49 output tokens of one branch end-to-end:
LN1, cross-attention (its queries vs all 196 keys/values of the other
modality), projection, residual, LN2, FFN, residual.

v2 design (latency-oriented rewrite of the feature-major baseline):
- Transposed attention: scores are computed keys-on-partitions as ONE
  [keys, 196] matmul per (head-quad, key-chunk) against a 4-block
  block-diagonal qT tile, so softmax Exp runs as 4 big ACT ops and attnV
  (v.T @ expT, M=128 covering two head-pairs) directly yields feature-major
  oT with no PE transposes or attT copies. The softmax denominators are a
  trailing ones-column matmul group in the same PSUM bank; 1/Z is computed
  as exp(-ln Z) on the ACT queue (a [1, n] single-lane DVE reciprocal is
  ~8 cycles/elem), broadcast via a 32-row ones outer product, and applied
  to the four valid diagonal blocks per chunk-group.
- Bias algebra: k-bias is softmax-invariant (dropped); v-bias is folded into
  the projection bias (softmax rows sum to 1); q-bias and both fp8 32x
  weight scales fold into the Exp scale (1/(16*1024)).
- Both LayerNorm rstds use exp(-0.5*ln(var+eps)) so every table-based ACT op
  (softmax Exp, 1/Z, LN rstd) lives in the natural_log_exp_and_others table;
  a compile-pass override restricts table choice so the whole kernel loads
  exactly 2 ACT tables (nl_exp at start, gelu prefetched after LN2 rstd).
- LN2 avoids a separate residual pass: t2 = pp/1024 + (nv - mean2-bcast) is
  rv - mean2 in one op, and var2 = mean(t2^2) with the 1/N folded into the
  Ln's scale operand. LN2's mean uses partial sums from nv (early) plus
  colsum(Wp) @ oT8 so it closes right after attention.
- FFN1 matmuls alternate PSUM banks (tile-granular WAR tracking otherwise
  serializes matmul->GELU->matmul); FFN2 accumulates per g8 pair; the
  projection runs plain-fp8 (FWL) per oT8 chunk-group so it starts before
  the second half of attention finishes.
- Inputs ship as 6 blobs on 2 HWDGE queues (sync/scalar), hot data first,
  the 512KB FFN blob last so it cannot starve the attention-path blobs.
- PSUM accumulation-group rule honored throughout: one pending group per
  2KB bank (LN1 packs [x | x^2] into a single group; Mb2 sums its two mean
  rows on DVE instead of accumulating across the attention span).
"""
import sys
if '/opt/trn_rl_repo' not in sys.path:
    sys.path.insert(0, '/opt/trn_rl_repo')

import os
import types
import numpy as np
import ml_dtypes

V2_SCORES = os.environ.get('V2_SCORES', 'bd')   # rt (row-tiled) | bd (blockdiag)
V2_RSTD = os.environ.get('V2_RSTD', 'lnexp')    # lnexp | sqrt
V2_DMA = os.environ.get('V2_DMA', 'multi')      # multi | sync

BF = ml_dtypes.bfloat16
F8 = ml_dtypes.float8_e4m3fn
N, EMB, H, DH, HID = 196, 256, 8, 32, 1024
T = 49            # tokens per core
EPS, SCALE = 1e-5, 16.0
WS = 32.0         # host-side fp8 weight scale
P = 128
NCORES = 8
TOKC = ((0, 128), (128, 68))   # token chunks of the 196 keys/values

CQ = 153   # blobQ: wq8i(128) | xq8(25)
CK = 226   # blobK: wk8i(128) | xkv8(98)
CA = 81    # blobA: aux(32)   | xq_bf(49)
CV = 256   # blobV: wv8(128)  | xkvi(128)
CP = 128   # blobP: wp8i(128)
CF = 1024  # blobF: w18i(512) | w28i(512)

_CACHE = {}


def _patched_atl(self):
    """Restrict the ACT-table choice to the two real tables that jointly
    cover every function used (exp/ln/identity/square in
    natural_log_exp_and_others; gelu/identity in gelu_and_others), so the
    insertion pass emits exactly one load per table instead of thrashing
    between exp/sqrt/gelu tables."""
    import concourse.mybir as mybir
    from concourse.hw_specs import get_activation_tables
    import bass_rust as _bass_rust
    has_activation = any(
        isinstance(i, mybir.InstActivation)
        for b in self.main_func.blocks for i in b.instructions)
    if not has_activation:
        return
    tabs = get_activation_tables(self.m.arch)
    if V2_RSTD == 'lnexp':
        keep = {'natural_log_exp_and_others', 'gelu_and_others'}
    else:
        keep = {'exp_and_others', 'sqrt_and_others', 'gelu_and_others'}
    tables = [(k, (v if k in keep else set())) for k, v in tabs.items()]
    _bass_rust.insert_act_table_loads(self, tables)


# ---------------------------------------------------------------- bass build
def build_bass():
    import concourse.bacc as bacc
    import concourse.mybir as mybir
    import concourse.tile as tile

    f32 = mybir.dt.float32
    bf16 = mybir.dt.bfloat16
    f8 = mybir.dt.float8e4
    AF = mybir.ActivationFunctionType
    OP = mybir.AluOpType
    DR = mybir.MatmulPerfMode.DoubleRowSwInterleave

    nc = bacc.Bacc("TRN2", target_bir_lowering=False)
    nc.insert_act_table_loads = types.MethodType(_patched_atl, nc)

    bQ_d = nc.dram_tensor("blobQ", [P, CQ], f32, kind="ExternalInput")
    bK_d = nc.dram_tensor("blobK", [P, CK], f32, kind="ExternalInput")
    bA_d = nc.dram_tensor("blobA", [P, CA], f32, kind="ExternalInput")
    bV_d = nc.dram_tensor("blobV", [P, CV], f32, kind="ExternalInput")
    bP_d = nc.dram_tensor("blobP", [P, CP], f32, kind="ExternalInput")
    bF_d = nc.dram_tensor("blobF", [P, CF], f32, kind="ExternalInput")
    out_d = nc.dram_tensor("out", [P, 2 * T], f32, kind="ExternalOutput")

    with tile.TileContext(nc) as tc:
        with (
            tc.tile_pool(name="const", bufs=1) as cpool,
            tc.tile_pool(name="act", bufs=1) as apool,
            tc.tile_pool(name="pwide", bufs=2, space="PSUM") as pwide,
            tc.tile_pool(name="psc", bufs=3, space="PSUM") as psc,
            tc.tile_pool(name="po", bufs=2, space="PSUM") as po,
            tc.tile_pool(name="psh", bufs=1, space="PSUM") as psh,
        ):
            # shared PSUM bank: broadcast tiles + stat rows at fixed cols
            # (build-order discipline keeps the tile-granular deps correct)
            shb = psh.tile([P, 512], f32, tag="shb")
            MB1, MB2, LNP, MSNV, MSUM, SSQ2 = 0, 98, 196, 294, 343, 392

            # ---- constants / scratch (no input deps; run during DMA wait)
            ones_row = cpool.tile([1, P], bf16, tag="ones_row")
            nc.vector.memset(ones_row[:], 1.0)
            ones_r32 = cpool.tile([1, 64], f32, tag="ones_r32")
            nc.vector.memset(ones_r32[:], 1.0)
            ones_col = cpool.tile([P, 1], bf16, tag="ones_col")
            nc.vector.memset(ones_col[:], 1.0)
            ones1024 = cpool.tile([P, 1], bf16, tag="ones1024")
            nc.vector.memset(ones1024[:], 1024.0)
            epst = cpool.tile([1, 1], f32, tag="epst")
            nc.vector.memset(epst[:], EPS)
            dums = cpool.tile([1, 2], f32, tag="dums")


            # ---- input DMAs: sync / scalar / gpsimd queues in parallel
            # (tried issuing bQ/bK on the earlier-available gpsimd SWDGE
            # queue: measured 2-4us SLOWER — SWDGE transfer setup loses
            # more than the earlier issue gains; HWDGE queues it is)
            dma2 = nc.scalar if V2_DMA == 'multi' else nc.sync
            bQ = cpool.tile([P, CQ], f32, tag="bQ")
            nc.sync.dma_start(bQ[:], bQ_d[:, :])
            bA = cpool.tile([P, CA], f32, tag="bA")
            dma2.dma_start(bA[:], bA_d[:, :])
            # bK rides the near-idle scalar queue (only tiny bA ahead of
            # it) so the k matmuls stop waiting ~1us behind the sync rings
            bK = cpool.tile([P, CK], f32, tag="bK")
            dma2.dma_start(bK[:], bK_d[:, :])
            bV = cpool.tile([P, CV], f32, tag="bV")
            nc.sync.dma_start(bV[:], bV_d[:, :])
            bP = cpool.tile([P, CP], f32, tag="bP")
            nc.sync.dma_start(bP[:], bP_d[:, :])
            # bF last on sync: its 512KB must not race the hot blobs into
            # the DMA-engine FIFOs (w1/w2 aren't needed until ~15us)
            bF = cpool.tile([P, CF], f32, tag="bF")
            nc.sync.dma_start(bF[:], bF_d[:, :])

            # dummy exp: pulls the nl_exp ACT table load onto the queue
            # early; gated on bA so it sits after the scalar-queue DMA issues
            nc.scalar.activation(dums[0:1, 0:1], bA[0:1, 0:1], AF.Exp,
                                 scale=0.0)

            # ---- views
            wq = bQ[:, 0:128].bitcast(f8).rearrange("p (b w) -> p b w", b=2)
            xq8 = bQ[:, 128:153].bitcast(f8)[:, 0:98].rearrange(
                "p (c t) -> p c t", c=2)
            wk = bK[:, 0:128].bitcast(f8).rearrange("p (b w) -> p b w", b=2)
            xkv = bK[:, 128:226].bitcast(f8)[:, 0:392].rearrange(
                "p (c t) -> p c t", c=2)
            aux = bA[:, 0:32]
            xq_bf = bA[:, 32:81].bitcast(bf16)[:, 0:98].rearrange(
                "p (c t) -> p c t", c=2)
            bq32_c = aux[:, 0:2]
            b1_c = aux[:, 2:10]
            ln1bp_c = aux[:, 10:12]
            ln2b_c = aux[:, 12:14]
            ln1w_c = aux[:, 14:16]
            ln2w_c = aux[:, 16:18]
            ln2bb2_c = aux[:, 18:20]
            wpcs32 = aux[:, 20:21].bitcast(f8)
            wv = bV[:, 0:128].bitcast(f8).rearrange("p (c m) -> p c m", c=2)
            xkvi = bV[:, 128:256].bitcast(f8)     # [128, 512] interleaved
            wpp = bP[:, 0:128].bitcast(f8).rearrange(
                "p (c b w) -> p c b w", c=2, b=2)
            w1 = bF[:, 0:512].bitcast(f8).rearrange("p (b w) -> p b w", b=8)
            w2 = bF[:, 512:1024].bitcast(f8).rearrange(
                "p (b i w) -> p b i w", b=2, i=4)

            # ---------------- q: dense fp8 DoubleRow, bias (32*bq) on cast
            qT = []
            if V2_SCORES == 'bd':
                for cg in range(2):
                    bd = apool.tile([P, 4 * T], bf16, tag=f"bd{cg}")
                    nc.vector.memset(bd[:], 0.0)
                    qT.append(bd)
            for mc in range(2):
                qp = pwide.tile([P, 512], f32, tag="pwide")
                nc.tensor.matmul(qp[:, 0:T], wq[:, mc], xq8[:],
                                 start=True, stop=True, perf_mode=DR)
                if V2_SCORES == 'bd':
                    # scatter 4 head rows into block-diag [128, 98] tiles,
                    # split across DVE and ACT queues
                    for hh in range(4):
                        dst = qT[mc][32 * hh:32 * hh + 32,
                                     49 * hh:49 * hh + T]
                        src = qp[32 * hh:32 * hh + 32, 0:T]
                        bcol = bq32_c[32 * hh:32 * hh + 32, mc:mc + 1]
                        if hh % 2 == 0:
                            nc.vector.tensor_scalar(dst, src, bcol, None,
                                                    op0=OP.add)
                        else:
                            nc.scalar.add(dst, src, bcol)
                else:
                    qt = apool.tile([P, T], bf16, tag=f"qT{mc}")
                    nc.vector.tensor_scalar(qt[:], qp[:, 0:T],
                                            bq32_c[:, mc:mc + 1], None,
                                            op0=OP.add)
                    qT.append(qt)

            # ---------------- k: feature-major 32*k, no bias
            kt = []
            for cg in range(2):
                pk = pwide.tile([P, 512], f32, tag="pwide")
                nc.tensor.matmul(pk[:, 0:N], wk[:, cg], xkv[:],
                                 start=True, stop=True, perf_mode=DR)
                k = apool.tile([P, N], bf16, tag=f"kT{cg}")
                if cg == 0:
                    nc.vector.tensor_copy(k[:], pk[:, 0:N])
                else:
                    nc.scalar.copy(k[:], pk[:, 0:N])
                kt.append(k)

            # ---------------- v: token-major 32*v plus ones column (softmax
            # denominator rides the attnV matmul as output row 64)
            va = []
            for tcx, (t0, tsz) in enumerate(TOKC):
                pv = pwide.tile([P, 512], f32, tag="pwide")
                nc.tensor.matmul(pv[:, 0:EMB],
                                 xkvi[:, 256 * tcx:256 * (tcx + 1)],
                                 wv[:], start=True, stop=True, perf_mode=DR)
                vat = apool.tile([P, 4, 64], bf16, tag=f"va{tcx}")
                nc.vector.tensor_copy(
                    vat[0:tsz, :, :],
                    pv[0:tsz, 0:EMB].rearrange("p (a b) -> p a b", a=4))
                va.append(vat)

            # ---------------- LN1 stats: sums/sumsqs via ones-col matmuls
            # pack [x | x^2] so mean+var ride ONE accumulation group (one
            # pending group per PSUM bank is a hardware constraint)
            xsq = apool.tile([P, 2, 2 * T], bf16, tag="xsq")
            nc.vector.tensor_copy(xsq[:, :, 0:T], xq_bf[:])
            nc.vector.tensor_tensor(xsq[:, :, T:2 * T], xq_bf[:], xq_bf[:],
                                    op=OP.mult)
            for kc in range(2):
                nc.tensor.matmul(shb[0:1, LNP:LNP + 2 * T], ones_col[:],
                                 xsq[:, kc], start=(kc == 0), stop=(kc == 1))
            mr1 = apool.tile([1, 2, T], bf16, tag="mr1")
            nc.vector.tensor_scalar(mr1[0:1, 0], shb[0:1, LNP:LNP + T],
                                    1.0 / EMB, None, op0=OP.mult)
            m1sq = apool.tile([1, T], f32, tag="m1sq")
            nc.vector.tensor_tensor(m1sq[:], mr1[0:1, 0], mr1[0:1, 0],
                                    op=OP.mult)
            d1 = apool.tile([1, T], f32, tag="d1")
            nc.vector.scalar_tensor_tensor(d1[:], shb[0:1, LNP + T:LNP + 2 * T],
                                           1.0 / EMB, m1sq[:],
                                           op0=OP.mult, op1=OP.subtract)
            def rstd_into(dst_row, d_ap, tagp, scale=1.0):
                """dst_row [1, T] bf16 = 1/sqrt(d_ap*scale + eps)."""
                if V2_RSTD == 'lnexp':
                    ldt = apool.tile([1, T], f32, tag=f"ld{tagp}")
                    nc.scalar.activation(ldt[:], d_ap, AF.Ln, scale=scale,
                                         bias=epst[0:1, 0:1])
                    nc.scalar.activation(dst_row, ldt[:], AF.Exp, scale=-0.5)
                else:
                    vst = apool.tile([1, T], f32, tag=f"vs{tagp}")
                    nc.scalar.activation(vst[:], d_ap, AF.Sqrt, scale=scale,
                                         bias=epst[0:1, 0:1])
                    rf = apool.tile([1, T], f32, tag=f"rf{tagp}")
                    nc.vector.reciprocal_approx_fast(rf[:], vst[:])
                    nc.vector.tensor_copy(dst_row, rf[:])

            rstd_into(mr1[0:1, 1], d1[:], '1')
            nc.tensor.matmul(shb[:, MB1:MB1 + 2 * T], ones_row[:],
                             mr1[:].rearrange("p c t -> p (c t)"),
                             start=True, stop=True)
            t1a = apool.tile([P, 2, T], f32, tag="t1a")
            u1 = apool.tile([P, 2, T], f32, tag="u1")
            nv_bf = apool.tile([P, 2, T], bf16, tag="nv")
            for kc in range(2):
                nc.vector.tensor_tensor(t1a[:, kc], xq_bf[:, kc],
                                        shb[:, MB1:MB1 + T], op=OP.subtract)
            for kc in range(2):
                nc.vector.scalar_tensor_tensor(u1[:, kc], t1a[:, kc],
                                               ln1w_c[:, kc:kc + 1],
                                               shb[:, MB1 + T:MB1 + 2 * T],
                                               op0=OP.mult, op1=OP.mult)
            for kc in range(2):
                nc.vector.tensor_scalar(nv_bf[:, kc], u1[:, kc],
                                        ln1bp_c[:, kc:kc + 1], None,
                                        op0=OP.add)

            # LN2 mean, nv part (closes well before attention finishes)
            for kc in range(2):
                nc.tensor.matmul(shb[0:1, MSNV:MSNV + T], ones1024[:],
                                 nv_bf[:, kc], start=(kc == 0), stop=(kc == 1))
            mr2 = apool.tile([1, 2, T], bf16, tag="mr2")
            nc.vector.tensor_scalar(mr2[0:1, 0], shb[0:1, MSNV:MSNV + T],
                                    1.0 / (1024.0 * EMB), None, op0=OP.mult)

            # ---------------- attention: transposed scores -> exp -> attnV
            # oT8 split per chunk-group so proj/msum start after pairs 0,1
            oT8 = []
            for cg in range(2):
                o8t = apool.tile([P, T], f8, tag=f"oT8_{cg}")
                oT8.append(o8t)
            expT = []
            for cg in range(2):
                ets = []
                for c, (t0, tsz) in enumerate(TOKC):
                    pss = psc.tile([P, 512], f32, tag="psc")
                    nc.tensor.matmul(pss[0:tsz, 0:4 * T],
                                     kt[cg][:, t0:t0 + tsz], qT[cg][:],
                                     start=True, stop=True)
                    et = apool.tile([P, 4 * T], bf16, tag=f"et{cg}_{c}")
                    nc.scalar.activation(et[0:tsz], pss[0:tsz, 0:4 * T],
                                         AF.Exp, scale=1.0 / (SCALE * 1024.0))
                    ets.append(et)
                expT.append(ets)
            for cg in range(2):
                ets = expT[cg]
                pot = po.tile([P, 512], f32, tag="po")
                # attnV for both pairs of this chunk-group in one M=128
                # matmul per key-chunk (diagonal blocks valid), then the
                # softmax sums as a second group in the same bank (legal:
                # the attnV group has closed)
                for c, (t0, tsz) in enumerate(TOKC):
                    nc.tensor.matmul(pot[0:P, 0:4 * T],
                                     va[c][0:tsz, 2 * cg:2 * cg + 2, :],
                                     ets[c][0:tsz, :],
                                     start=(c == 0), stop=(c == 1))
                for c, (t0, tsz) in enumerate(TOKC):
                    nc.tensor.matmul(pot[0:1, 256:256 + 4 * T],
                                     ones_col[0:tsz, :], ets[c][0:tsz, :],
                                     start=(c == 0), stop=(c == 1))
                # 1/Z for both pairs at once via exp(-ln Z) on ACT
                lz = apool.tile([1, 4 * T], f32, tag=f"lz{cg}")
                nc.scalar.activation(lz[:], pot[0:1, 256:256 + 4 * T], AF.Ln)
                rrow = apool.tile([1, 4 * T], bf16, tag=f"rr{cg}")
                nc.scalar.activation(rrow[:], lz[:], AF.Exp, scale=-1.0)
                pbb = psc.tile([P, 512], f32, tag="psc")
                nc.tensor.matmul(pbb[0:32, 0:4 * T], ones_row[0:1, 0:32],
                                 rrow[:], start=True, stop=True)
                rb = apool.tile([32, 4 * T], f32, tag=f"rb{cg}")
                if cg == 0:
                    nc.scalar.copy(rb[:], pbb[0:32, 0:4 * T])
                else:
                    nc.vector.tensor_copy(rb[:], pbb[0:32, 0:4 * T])
                for pl in range(2):      # pair-local within the chunk-group
                    for j in range(2):
                        rr0 = 64 * pl + 32 * j
                        cc0 = 98 * pl + 49 * j
                        nc.vector.tensor_tensor(
                            oT8[cg][rr0:rr0 + 32, :],
                            pot[rr0:rr0 + 32, cc0:cc0 + T],
                            rb[0:32, cc0:cc0 + T],
                            op=OP.mult)

            # ---------------- LN2 mean, early: 1024*sum_f rv = ones1024@nv
            #                  + (32*colsum(Wp)) @ (32*oT)
            # LN2 mean, attention part: trails only the oT8 tiles
            for kc in range(2):
                nc.tensor.matmul(shb[0:1, MSUM:MSUM + T], wpcs32[:, kc:kc + 1],
                                 oT8[kc][:], start=(kc == 0), stop=(kc == 1))
            mo2 = apool.tile([1, T], bf16, tag="mo2")
            nc.vector.scalar_tensor_tensor(mo2[:], shb[0:1, MSUM:MSUM + T],
                                           1.0 / (1024.0 * EMB), mr2[0:1, 0],
                                           op0=OP.mult, op1=OP.add)
            nc.tensor.matmul(shb[:, MB2:MB2 + T], ones_row[:], mo2[:],
                             start=True, stop=True)
            # nv_mb = nv - mean2 broadcast, ready before proj lands; then
            # t2 = pp/1024 + nv_mb directly IS rv - mean2 (one STT), and
            # var2 = mean(t2^2) exactly
            nv_mb = apool.tile([P, 2, T], bf16, tag="nv_mb")
            for kc in range(2):
                nc.vector.tensor_tensor(nv_mb[:, kc], nv_bf[:, kc],
                                        shb[:, MB2:MB2 + T], op=OP.subtract)

            # ---------------- projection: plain fp8 (FWL), per-cg accumulate
            pp = pwide.tile([P, 512], f32, tag="pwide")
            for mc in range(2):
                for cg in range(2):
                    nc.tensor.matmul(pp[:, 50 * mc:50 * mc + T],
                                     wpp[:, cg, mc], oT8[cg][:],
                                     start=(cg == 0), stop=(cg == 1))
            pp3 = pp[:, 0:100].rearrange("p (c t) -> p c t", c=2)[:, :, 0:T]
            t2 = apool.tile([P, 2, T], f32, tag="t2")
            nc.vector.scalar_tensor_tensor(t2[:], pp3, 1.0 / 1024.0,
                                           nv_mb[:], op0=OP.mult, op1=OP.add)

            # ---------------- LN2 rstd from t2: rstd = exp(-.5 ln(ssq/N+eps))
            sq2 = apool.tile([P, 2, T], bf16, tag="sq2")
            nc.vector.tensor_tensor(sq2[:], t2[:], t2[:], op=OP.mult)
            for kc in range(2):
                nc.tensor.matmul(shb[0:1, SSQ2:SSQ2 + T], ones_col[:],
                                 sq2[:, kc], start=(kc == 0), stop=(kc == 1))
            rstd_into(mr2[0:1, 1], shb[0:1, SSQ2:SSQ2 + T], '2',
                      scale=1.0 / EMB)
            nc.tensor.matmul(shb[:, MB2 + T:MB2 + 2 * T], ones_row[:],
                             mr2[0:1, 1], start=True, stop=True)
            # lv8 = u2 cast straight to f8 (ln2b is folded into the FFN1
            # bias host-side), so the FFN starts 2 hops earlier; the bf16
            # residual copy + bias ride during the FFN
            lv8 = apool.tile([P, 2, T], f8, tag="lv8")
            for kc in range(2):
                nc.vector.scalar_tensor_tensor(lv8[:, kc], t2[:, kc],
                                               ln2w_c[:, kc:kc + 1],
                                               shb[:, MB2 + T:MB2 + 2 * T],
                                               op0=OP.mult, op1=OP.mult)
            u2bf = apool.tile([P, 2, T], bf16, tag="u2bf")
            for kc in range(2):
                nc.vector.scalar_tensor_tensor(u2bf[:, kc], t2[:, kc],
                                               ln2w_c[:, kc:kc + 1],
                                               shb[:, MB2 + T:MB2 + 2 * T],
                                               op0=OP.mult, op1=OP.mult)
            lvb2 = apool.tile([P, 2, T], bf16, tag="lvb2")
            for kc in range(2):
                nc.vector.tensor_scalar(lvb2[:, kc], u2bf[:, kc],
                                        ln2bb2_c[:, kc:kc + 1], None,
                                        op0=OP.add)
            # gelu-table prefetch, gated on LN2 rstd (last nl_exp-table op)
            gf = AF.Identity if _CACHE.get('sim_ident_gelu') else AF.Gelu
            nc.scalar.activation(dums[0:1, 1:2], mr2[0:1, 1, 0:1], gf)

            # ---------------- FFN fp8 DoubleRow, two-bank psum pipeline
            g8 = []
            for j in range(4):
                g8t = apool.tile([P, 2, T], f8, tag=f"g8_{j}")
                g8.append(g8t)
            for j in range(4):
                ph = psc.tile([P, 512], f32, tag="psc")
                for jj in range(2):
                    mc = 2 * j + jj
                    nc.tensor.matmul(ph[:, 50 * jj:50 * jj + T], w1[:, mc],
                                     lv8[:], start=True, stop=True,
                                     perf_mode=DR)
                    nc.scalar.activation(g8[j][:, jj],
                                         ph[:, 50 * jj:50 * jj + T],
                                         gf, scale=1.0 / WS,
                                         bias=b1_c[:, mc:mc + 1])
            for mc in range(2):
                pf = pwide.tile([P, 512], f32, tag="pwide")
                for i in range(4):
                    nc.tensor.matmul(pf[:, 0:T], w2[:, mc, i], g8[i][:],
                                     start=(i == 0), stop=(i == 3),
                                     perf_mode=DR)
                ot = apool.tile([P, T], f32, tag=f"out{mc}")
                nc.vector.scalar_tensor_tensor(ot[:], pf[:, 0:T],
                                               1.0 / WS, lvb2[:, mc],
                                               op0=OP.mult, op1=OP.add)
                eng = nc.sync if mc == 0 else nc.gpsimd
                eng.dma_start(out_d[:, mc * T:(mc + 1) * T], ot[:],
                              single_packet=True)

    nc.compile()
    return nc


# ---------------------------------------------------------------- host side
def _reorder_qkv(W, b):
    W4 = np.asarray(W, np.float32).reshape(EMB, H, DH, 3)
    b4 = np.asarray(b, np.float32).reshape(H, DH, 3)
    return ([np.ascontiguousarray(W4[:, :, :, i].reshape(EMB, EMB)) for i in range(3)],
            [np.ascontiguousarray(b4[:, :, i].reshape(EMB)) for i in range(3)])


def _pack_w8(w):
    """(K, M) f32 -> x32-scaled partition-major (128, K//128 * M) f8 block."""
    w = np.asarray(w, np.float32) * WS
    k, m = w.shape
    c = k // P
    return np.transpose(w.reshape(c, P, m), (1, 0, 2)).reshape(P, c * m).astype(F8)


def _ileave(pair3):
    """[P, 2, m] -> [P, 2m]: A/B column-interleaved, columns reversed
    (DoubleRowSwInterleave weight layout)."""
    blk = pair3[:, :, ::-1]
    return np.ascontiguousarray(np.transpose(blk, (0, 2, 1)).reshape(P, -1))


def _pack_w8i(w, mc_cols, mc_major=False):
    """(K, M) f32 -> x32-scaled SwInterleave layout: one [P, 2*mc_cols]
    interleaved block per (k-tile pair, M-chunk)."""
    w = np.asarray(w, np.float32) * WS
    k, m = w.shape
    c = k // P
    w3 = np.transpose(w.reshape(c, P, m), (1, 0, 2)).astype(F8)  # [P, c, m]
    blk = [[_ileave(w3[:, 2 * i:2 * i + 2, j0:j0 + mc_cols])
            for j0 in range(0, m, mc_cols)] for i in range(c // 2)]
    if mc_major:
        out = [blk[i][j] for j in range(len(blk[0])) for i in range(len(blk))]
    else:
        out = [blk[i][j] for i in range(len(blk)) for j in range(len(blk[0]))]
    return np.concatenate(out, axis=1)


def _pack_x(x):
    """(tokens, 256) -> (128, 2*tokens) f32 partition-major transposed."""
    xt = np.ascontiguousarray(np.asarray(x, np.float32).T)       # (256, t)
    t = xt.shape[1]
    return np.transpose(xt.reshape(2, P, t), (1, 0, 2)).reshape(P, 2 * t)


def _f8cols(a):
    """(128, n) f8 array -> zero-padded f32-col view (n_f32 = ceil(n/4))."""
    n = a.shape[1]
    pad = (-n) % 4
    if pad:
        a = np.concatenate([a, np.zeros((P, pad), F8)], axis=1)
    return np.ascontiguousarray(a).view(np.float32)


def _bfcols(a):
    """(128, n) bf16 array -> zero-padded f32-col view."""
    n = a.shape[1]
    pad = (-n) % 2
    if pad:
        a = np.concatenate([a, np.zeros((P, pad), BF)], axis=1)
    return np.ascontiguousarray(a).view(np.float32)


def _cols(v):
    """(256,) -> (128, 2) natural feature chunks."""
    return np.ascontiguousarray(np.asarray(v, np.float32).reshape(2, P).T)


def make_in_maps(inputs):
    inp = {k: np.asarray(v, np.float32) for k, v in inputs.items()}
    qkv_v = _reorder_qkv(inp['Wqkv_v'], inp['bqkv_v'])
    qkv_i = _reorder_qkv(inp['Wqkv_i'], inp['bqkv_i'])
    out = []
    for core in range(NCORES):
        r0 = (core % 4) * T
        if core // 4 == 0:  # vis output: vis queries, ir keys/values
            x_own, x_oth = inp['vis_emb'][0], inp['ir_emb'][0]
            wq, bq = qkv_v[0][0], qkv_v[1][0]
            wk = qkv_i[0][1]
            wv, bv = qkv_i[0][2], qkv_i[1][2]
            wp, bp = inp['Wp_v'], inp['bp_v']
            lnp = (inp['ln1v_w'], inp['ln1v_b'], inp['ln2v_w'], inp['ln2v_b'])
            w1, b1, w2, b2 = inp['W1v'], inp['b1v'], inp['W2v'], inp['b2v']
        else:               # ir output: ir queries, vis kv
            x_own, x_oth = inp['ir_emb'][0], inp['vis_emb'][0]
            wq, bq = qkv_i[0][0], qkv_i[1][0]
            wk = qkv_v[0][1]
            wv, bv = qkv_v[0][2], qkv_v[1][2]
            wp, bp = inp['Wp_i'], inp['bp_i']
            lnp = (inp['ln1i_w'], inp['ln1i_b'], inp['ln2i_w'], inp['ln2i_b'])
            w1, b1, w2, b2 = inp['W1i'], inp['b1i'], inp['W2i'], inp['b2i']

        # fold v-bias into the projection bias: softmax rows sum to 1
        bp_f = np.asarray(bp, np.float32) + np.asarray(bv, np.float32) @ np.asarray(wp, np.float32)

        # ln2-bias folded into the FFN1 bias: gelu((u2+ln2b)@W1 + b1)
        # = gelu(u2@W1 + (ln2b@W1 + b1))
        b1_f = (np.asarray(b1, np.float32)
                + np.asarray(lnp[3], np.float32) @ np.asarray(w1, np.float32))
        aux = np.zeros((P, 32), np.float32)
        aux[:, 0:2] = _cols(np.asarray(bq, np.float32) * WS)
        aux[:, 2:10] = b1_f.reshape(8, P).T
        aux[:, 10:12] = _cols(np.asarray(lnp[1], np.float32) + bp_f)
        aux[:, 12:14] = _cols(lnp[3])
        aux[:, 14:16] = _cols(lnp[0])
        aux[:, 16:18] = _cols(lnp[2])
        aux[:, 18:20] = _cols(np.asarray(lnp[3], np.float32)
                              + np.asarray(b2, np.float32))
        wpcs = np.zeros((P, 4), F8)
        wpcs[:, 0:2] = (np.asarray(wp, np.float32).sum(axis=1) * WS
                        ).reshape(2, P).T.astype(F8)
        aux[:, 20] = wpcs.view(np.float32)[:, 0]

        xq_f = _pack_x(x_own[r0:r0 + T])            # [128, 98] f32
        xkv_f = _pack_x(x_oth)                      # [128, 392] f32
        xkv3 = np.zeros((P, 2, 256), F8)
        xkv3[:, :, 0:N] = xkv_f.astype(F8).reshape(P, 2, N)
        xkvi = np.concatenate(
            [_ileave(xkv3[:, :, 0:128]), _ileave(xkv3[:, :, 128:256])], axis=1)

        blobQ = np.concatenate([
            _f8cols(_pack_w8i(wq, P)),                  # 128
            _f8cols(xq_f.astype(F8)),                   # 25
        ], axis=1)
        blobK = np.concatenate([
            _f8cols(_pack_w8i(wk, P)),                  # 128
            _f8cols(xkv_f.astype(F8)),                  # 98
        ], axis=1)
        blobA = np.concatenate([
            aux,                                        # 32
            _bfcols(xq_f.astype(BF)),                   # 49
        ], axis=1)
        blobV = np.concatenate([
            _f8cols(_pack_w8(wv)),                      # 128
            _f8cols(xkvi),                              # 128
        ], axis=1)
        blobP = _f8cols(_pack_w8(wp))                   # 128
        blobF = np.concatenate([
            _f8cols(_pack_w8i(w1, P)),                  # 512
            _f8cols(_pack_w8i(w2, P, mc_major=True)),   # 512
        ], axis=1)
        out.append({
            'blobQ': np.ascontiguousarray(blobQ),
            'blobK': np.ascontiguousarray(blobK),
            'blobA': np.ascontiguousarray(blobA),
            'blobV': np.ascontiguousarray(blobV),
            'blobP': np.ascontiguousarray(blobP),
            'blobF': np.ascontiguousarray(blobF),
        })
    return out


def _recon(x):
    x = x.reshape(14, 14, 16, 16)
    x = np.transpose(x, (2, 3, 0, 1))
    return x.reshape(1, 1, 224, 224)


def assemble(core_outs):
    # core out [128, 98] f32 -> [49 tokens, 256 feats]
    toks = [np.transpose(np.asarray(o, np.float32).reshape(P, 2, T),
                         (2, 1, 0)).reshape(T, EMB) for o in core_outs]
    ov = np.concatenate(toks[0:4], axis=0)
    oi = np.concatenate(toks[4:8], axis=0)
    return np.concatenate([_recon(oi), _recon(ov)], axis=1).astype(np.float32)


def get_nc():
    if 'nc' not in _CACHE:
        _CACHE['nc'] = build_bass()
    return _CACHE['nc']


def kernel(**inputs):
    from concourse import bass_utils
    nc = get_nc()
    in_maps = make_in_maps(inputs)
    res = bass_utils.run_bass_kernel_spmd(nc, in_maps, core_ids=list(range(NCORES)))
    outs = [np.asarray(r['out'], np.float32) for r in res.results]
    return assemble(outs)


# revision 62
# speedup vs baseline: 1.0257x; 1.0257x over previous
"""CrossViT fused block on 8 TRN2 NeuronCores.

Sharding: 2 branches (vis-output / ir-output) x 4-way token split -> 8 cores,
no collectives. Each core computes 49 output tokens of one branch end-to-end:
LN1, cross-attention (its queries vs all 196 keys/values of the other
modality), projection, residual, LN2, FFN, residual.

v2 design (latency-oriented rewrite of the feature-major baseline):
- Transposed attention: scores are computed keys-on-partitions as ONE
  [keys, 196] matmul per (head-quad, key-chunk) against a 4-block
  block-diagonal qT tile, so softmax Exp runs as 4 big ACT ops and attnV
  (v.T @ expT, M=128 covering two head-pairs) directly yields feature-major
  oT with no PE transposes or attT copies. The softmax denominators are a
  trailing ones-column matmul group in the same PSUM bank; 1/Z is computed
  as exp(-ln Z) on the ACT queue (a [1, n] single-lane DVE reciprocal is
  ~8 cycles/elem), broadcast via a 32-row ones outer product, and applied
  to the four valid diagonal blocks per chunk-group.
- Bias algebra: k-bias is softmax-invariant (dropped); v-bias is folded into
  the projection bias (softmax rows sum to 1); q-bias and both fp8 32x
  weight scales fold into the Exp scale (1/(16*1024)).
- Both LayerNorm rstds use exp(-0.5*ln(var+eps)) so every table-based ACT op
  (softmax Exp, 1/Z, LN rstd) lives in the natural_log_exp_and_others table;
  a compile-pass override restricts table choice so the whole kernel loads
  exactly 2 ACT tables (nl_exp at start, gelu prefetched after LN2 rstd).
- LN2 avoids a separate residual pass: t2 = pp/1024 + (nv - mean2-bcast) is
  rv - mean2 in one op, and var2 = mean(t2^2) with the 1/N folded into the
  Ln's scale operand. LN2's mean uses partial sums from nv (early) plus
  colsum(Wp) @ oT8 so it closes right after attention.
- FFN1 matmuls alternate PSUM banks (tile-granular WAR tracking otherwise
  serializes matmul->GELU->matmul); FFN2 accumulates per g8 pair; the
  projection runs plain-fp8 (FWL) per oT8 chunk-group so it starts before
  the second half of attention finishes.
- Inputs ship as 6 blobs on 2 HWDGE queues (sync/scalar), hot data first,
  the 512KB FFN blob last so it cannot starve the attention-path blobs.
- PSUM accumulation-group rule honored throughout: one pending group per
  2KB bank (LN1 packs [x | x^2] into a single group; Mb2 sums its two mean
  rows on DVE instead of accumulating across the attention span).
"""
import sys
if '/opt/trn_rl_repo' not in sys.path:
    sys.path.insert(0, '/opt/trn_rl_repo')

import os
import types
import numpy as np
import ml_dtypes

V2_SCORES = os.environ.get('V2_SCORES', 'bd')   # rt (row-tiled) | bd (blockdiag)
V2_RSTD = os.environ.get('V2_RSTD', 'lnexp')    # lnexp | sqrt
V2_DMA = os.environ.get('V2_DMA', 'multi')      # multi | sync

BF = ml_dtypes.bfloat16
F8 = ml_dtypes.float8_e4m3fn
N, EMB, H, DH, HID = 196, 256, 8, 32, 1024
T = 49            # tokens per core
EPS, SCALE = 1e-5, 16.0
WS = 32.0         # host-side fp8 weight scale
P = 128
NCORES = 8
TOKC = ((0, 128), (128, 68))   # token chunks of the 196 keys/values

CQ = 153   # blobQ: wq8i(128) | xq8(25)
CK = 226   # blobK: wk8i(128) | xkv8(98)
CA = 81    # blobA: aux(32)   | xq_bf(49)
CV = 256   # blobV: wv8(128)  | xkvi(128)
CP = 128   # blobP: wp8i(128)
CF = 1024  # blobF: w18i(512) | w28i(512)

_CACHE = {}


def _patched_atl(self):
    """Restrict the ACT-table choice to the two real tables that jointly
    cover every function used (exp/ln/identity/square in
    natural_log_exp_and_others; gelu/identity in gelu_and_others), so the
    insertion pass emits exactly one load per table instead of thrashing
    between exp/sqrt/gelu tables."""
    import concourse.mybir as mybir
    from concourse.hw_specs import get_activation_tables
    import bass_rust as _bass_rust
    has_activation = any(
        isinstance(i, mybir.InstActivation)
        for b in self.main_func.blocks for i in b.instructions)
    if not has_activation:
        return
    tabs = get_activation_tables(self.m.arch)
    if V2_RSTD == 'lnexp':
        keep = {'natural_log_exp_and_others', 'gelu_and_others'}
    else:
        keep = {'exp_and_others', 'sqrt_and_others', 'gelu_and_others'}
    tables = [(k, (v if k in keep else set())) for k, v in tabs.items()]
    _bass_rust.insert_act_table_loads(self, tables)


# ---------------------------------------------------------------- bass build
def build_bass():
    import concourse.bacc as bacc
    import concourse.mybir as mybir
    import concourse.tile as tile

    f32 = mybir.dt.float32
    bf16 = mybir.dt.bfloat16
    f8 = mybir.dt.float8e4
    AF = mybir.ActivationFunctionType
    OP = mybir.AluOpType
    DR = mybir.MatmulPerfMode.DoubleRowSwInterleave

    nc = bacc.Bacc("TRN2", target_bir_lowering=False)
    nc.insert_act_table_loads = types.MethodType(_patched_atl, nc)

    bQ_d = nc.dram_tensor("blobQ", [P, CQ], f32, kind="ExternalInput")
    bK_d = nc.dram_tensor("blobK", [P, CK], f32, kind="ExternalInput")
    bA_d = nc.dram_tensor("blobA", [P, CA], f32, kind="ExternalInput")
    bV_d = nc.dram_tensor("blobV", [P, CV], f32, kind="ExternalInput")
    bP_d = nc.dram_tensor("blobP", [P, CP], f32, kind="ExternalInput")
    bF_d = nc.dram_tensor("blobF", [P, CF], f32, kind="ExternalInput")
    out_d = nc.dram_tensor("out", [P, 2 * T], f32, kind="ExternalOutput")

    with tile.TileContext(nc) as tc:
        with (
            tc.tile_pool(name="const", bufs=1) as cpool,
            tc.tile_pool(name="act", bufs=1) as apool,
            tc.tile_pool(name="pwide", bufs=2, space="PSUM") as pwide,
            tc.tile_pool(name="psc", bufs=3, space="PSUM") as psc,
            tc.tile_pool(name="po", bufs=2, space="PSUM") as po,
            tc.tile_pool(name="psh", bufs=1, space="PSUM") as psh,
        ):
            # shared PSUM bank: broadcast tiles + stat rows at fixed cols
            # (build-order discipline keeps the tile-granular deps correct)
            shb = psh.tile([P, 512], f32, tag="shb")
            MB1, MB2, LNP, MSNV, MSUM, SSQ2 = 0, 98, 196, 294, 343, 392

            # ---- constants / scratch (no input deps; run during DMA wait)
            ones_row = cpool.tile([1, P], bf16, tag="ones_row")
            nc.vector.memset(ones_row[:], 1.0)
            ones_r32 = cpool.tile([1, 64], f32, tag="ones_r32")
            nc.vector.memset(ones_r32[:], 1.0)
            ones_col = cpool.tile([P, 1], bf16, tag="ones_col")
            nc.vector.memset(ones_col[:], 1.0)
            ones1024 = cpool.tile([P, 1], bf16, tag="ones1024")
            nc.vector.memset(ones1024[:], 1024.0)
            epst = cpool.tile([1, 1], f32, tag="epst")
            nc.vector.memset(epst[:], EPS)
            dums = cpool.tile([1, 2], f32, tag="dums")


            # ---- input DMAs: sync / scalar / gpsimd queues in parallel
            # (tried issuing bQ/bK on the earlier-available gpsimd SWDGE
            # queue: measured 2-4us SLOWER — SWDGE transfer setup loses
            # more than the earlier issue gains; HWDGE queues it is)
            dma2 = nc.scalar if V2_DMA == 'multi' else nc.sync
            bQ = cpool.tile([P, CQ], f32, tag="bQ")
            nc.sync.dma_start(bQ[:], bQ_d[:, :])
            bA = cpool.tile([P, CA], f32, tag="bA")
            dma2.dma_start(bA[:], bA_d[:, :])
            bK = cpool.tile([P, CK], f32, tag="bK")
            nc.sync.dma_start(bK[:], bK_d[:, :])
            bV = cpool.tile([P, CV], f32, tag="bV")
            dma2.dma_start(bV[:], bV_d[:, :])
            bP = cpool.tile([P, CP], f32, tag="bP")
            nc.sync.dma_start(bP[:], bP_d[:, :])
            # bF last on sync: its 512KB must not race the hot blobs into
            # the DMA-engine FIFOs (w1/w2 aren't needed until ~15us)
            bF = cpool.tile([P, CF], f32, tag="bF")
            nc.sync.dma_start(bF[:], bF_d[:, :])

            # dummy exp: pulls the nl_exp ACT table load onto the queue
            # early; gated on bA so it sits after the scalar-queue DMA issues
            nc.scalar.activation(dums[0:1, 0:1], bA[0:1, 0:1], AF.Exp,
                                 scale=0.0)

            # ---- views
            wq = bQ[:, 0:128].bitcast(f8).rearrange("p (b w) -> p b w", b=2)
            xq8 = bQ[:, 128:153].bitcast(f8)[:, 0:98].rearrange(
                "p (c t) -> p c t", c=2)
            wk = bK[:, 0:128].bitcast(f8).rearrange("p (b w) -> p b w", b=2)
            xkv = bK[:, 128:226].bitcast(f8)[:, 0:392].rearrange(
                "p (c t) -> p c t", c=2)
            aux = bA[:, 0:32]
            xq_bf = bA[:, 32:81].bitcast(bf16)[:, 0:98].rearrange(
                "p (c t) -> p c t", c=2)
            bq32_c = aux[:, 0:2]
            b1_c = aux[:, 2:10]
            ln1bp_c = aux[:, 10:12]
            ln2b_c = aux[:, 12:14]
            ln1w_c = aux[:, 14:16]
            ln2w_c = aux[:, 16:18]
            ln2bb2_c = aux[:, 18:20]
            wpcs32 = aux[:, 20:21].bitcast(f8)
            wv = bV[:, 0:128].bitcast(f8).rearrange("p (c m) -> p c m", c=2)
            xkvi = bV[:, 128:256].bitcast(f8)     # [128, 512] interleaved
            wpp = bP[:, 0:128].bitcast(f8).rearrange(
                "p (c b w) -> p c b w", c=2, b=2)
            w1 = bF[:, 0:512].bitcast(f8).rearrange("p (b w) -> p b w", b=8)
            w2 = bF[:, 512:1024].bitcast(f8).rearrange(
                "p (b i w) -> p b i w", b=2, i=4)

            # ---------------- q: dense fp8 DoubleRow, bias (32*bq) on cast
            qT = []
            if V2_SCORES == 'bd':
                for cg in range(2):
                    bd = apool.tile([P, 4 * T], bf16, tag=f"bd{cg}")
                    nc.vector.memset(bd[:], 0.0)
                    qT.append(bd)
            for mc in range(2):
                qp = pwide.tile([P, 512], f32, tag="pwide")
                nc.tensor.matmul(qp[:, 0:T], wq[:, mc], xq8[:],
                                 start=True, stop=True, perf_mode=DR)
                if V2_SCORES == 'bd':
                    # scatter 4 head rows into block-diag [128, 98] tiles,
                    # split across DVE and ACT queues
                    for hh in range(4):
                        dst = qT[mc][32 * hh:32 * hh + 32,
                                     49 * hh:49 * hh + T]
                        src = qp[32 * hh:32 * hh + 32, 0:T]
                        bcol = bq32_c[32 * hh:32 * hh + 32, mc:mc + 1]
                        if hh % 2 == 0:
                            nc.vector.tensor_scalar(dst, src, bcol, None,
                                                    op0=OP.add)
                        else:
                            nc.scalar.add(dst, src, bcol)
                else:
                    qt = apool.tile([P, T], bf16, tag=f"qT{mc}")
                    nc.vector.tensor_scalar(qt[:], qp[:, 0:T],
                                            bq32_c[:, mc:mc + 1], None,
                                            op0=OP.add)
                    qT.append(qt)

            # ---------------- k: feature-major 32*k, no bias
            kt = []
            for cg in range(2):
                pk = pwide.tile([P, 512], f32, tag="pwide")
                nc.tensor.matmul(pk[:, 0:N], wk[:, cg], xkv[:],
                                 start=True, stop=True, perf_mode=DR)
                k = apool.tile([P, N], bf16, tag=f"kT{cg}")
                if cg == 0:
                    nc.vector.tensor_copy(k[:], pk[:, 0:N])
                else:
                    nc.scalar.copy(k[:], pk[:, 0:N])
                kt.append(k)

            # ---------------- v: token-major 32*v plus ones column (softmax
            # denominator rides the attnV matmul as output row 64)
            va = []
            for tcx, (t0, tsz) in enumerate(TOKC):
                pv = pwide.tile([P, 512], f32, tag="pwide")
                nc.tensor.matmul(pv[:, 0:EMB],
                                 xkvi[:, 256 * tcx:256 * (tcx + 1)],
                                 wv[:], start=True, stop=True, perf_mode=DR)
                vat = apool.tile([P, 4, 64], bf16, tag=f"va{tcx}")
                nc.vector.tensor_copy(
                    vat[0:tsz, :, :],
                    pv[0:tsz, 0:EMB].rearrange("p (a b) -> p a b", a=4))
                va.append(vat)

            # ---------------- LN1 stats: sums/sumsqs via ones-col matmuls
            # pack [x | x^2] so mean+var ride ONE accumulation group (one
            # pending group per PSUM bank is a hardware constraint)
            xsq = apool.tile([P, 2, 2 * T], bf16, tag="xsq")
            nc.vector.tensor_copy(xsq[:, :, 0:T], xq_bf[:])
            nc.vector.tensor_tensor(xsq[:, :, T:2 * T], xq_bf[:], xq_bf[:],
                                    op=OP.mult)
            for kc in range(2):
                nc.tensor.matmul(shb[0:1, LNP:LNP + 2 * T], ones_col[:],
                                 xsq[:, kc], start=(kc == 0), stop=(kc == 1))
            mr1 = apool.tile([1, 2, T], bf16, tag="mr1")
            nc.vector.tensor_scalar(mr1[0:1, 0], shb[0:1, LNP:LNP + T],
                                    1.0 / EMB, None, op0=OP.mult)
            m1sq = apool.tile([1, T], f32, tag="m1sq")
            nc.vector.tensor_tensor(m1sq[:], mr1[0:1, 0], mr1[0:1, 0],
                                    op=OP.mult)
            d1 = apool.tile([1, T], f32, tag="d1")
            nc.vector.scalar_tensor_tensor(d1[:], shb[0:1, LNP + T:LNP + 2 * T],
                                           1.0 / EMB, m1sq[:],
                                           op0=OP.mult, op1=OP.subtract)
            def rstd_into(dst_row, d_ap, tagp, scale=1.0):
                """dst_row [1, T] bf16 = 1/sqrt(d_ap*scale + eps)."""
                if V2_RSTD == 'lnexp':
                    ldt = apool.tile([1, T], f32, tag=f"ld{tagp}")
                    nc.scalar.activation(ldt[:], d_ap, AF.Ln, scale=scale,
                                         bias=epst[0:1, 0:1])
                    nc.scalar.activation(dst_row, ldt[:], AF.Exp, scale=-0.5)
                else:
                    vst = apool.tile([1, T], f32, tag=f"vs{tagp}")
                    nc.scalar.activation(vst[:], d_ap, AF.Sqrt, scale=scale,
                                         bias=epst[0:1, 0:1])
                    rf = apool.tile([1, T], f32, tag=f"rf{tagp}")
                    nc.vector.reciprocal_approx_fast(rf[:], vst[:])
                    nc.vector.tensor_copy(dst_row, rf[:])

            rstd_into(mr1[0:1, 1], d1[:], '1')
            nc.tensor.matmul(shb[:, MB1:MB1 + 2 * T], ones_row[:],
                             mr1[:].rearrange("p c t -> p (c t)"),
                             start=True, stop=True)
            t1a = apool.tile([P, 2, T], f32, tag="t1a")
            u1 = apool.tile([P, 2, T], f32, tag="u1")
            nv_bf = apool.tile([P, 2, T], bf16, tag="nv")
            for kc in range(2):
                nc.vector.tensor_tensor(t1a[:, kc], xq_bf[:, kc],
                                        shb[:, MB1:MB1 + T], op=OP.subtract)
            for kc in range(2):
                nc.vector.scalar_tensor_tensor(u1[:, kc], t1a[:, kc],
                                               ln1w_c[:, kc:kc + 1],
                                               shb[:, MB1 + T:MB1 + 2 * T],
                                               op0=OP.mult, op1=OP.mult)
            for kc in range(2):
                nc.vector.tensor_scalar(nv_bf[:, kc], u1[:, kc],
                                        ln1bp_c[:, kc:kc + 1], None,
                                        op0=OP.add)

            # LN2 mean, nv part (closes well before attention finishes)
            for kc in range(2):
                nc.tensor.matmul(shb[0:1, MSNV:MSNV + T], ones1024[:],
                                 nv_bf[:, kc], start=(kc == 0), stop=(kc == 1))
            mr2 = apool.tile([1, 2, T], bf16, tag="mr2")
            nc.vector.tensor_scalar(mr2[0:1, 0], shb[0:1, MSNV:MSNV + T],
                                    1.0 / (1024.0 * EMB), None, op0=OP.mult)

            # ---------------- attention: transposed scores -> exp -> attnV
            # oT8 split per chunk-group so proj/msum start after pairs 0,1
            oT8 = []
            for cg in range(2):
                o8t = apool.tile([P, T], f8, tag=f"oT8_{cg}")
                oT8.append(o8t)
            expT = []
            for cg in range(2):
                ets = []
                for c, (t0, tsz) in enumerate(TOKC):
                    pss = psc.tile([P, 512], f32, tag="psc")
                    nc.tensor.matmul(pss[0:tsz, 0:4 * T],
                                     kt[cg][:, t0:t0 + tsz], qT[cg][:],
                                     start=True, stop=True)
                    et = apool.tile([P, 4 * T], bf16, tag=f"et{cg}_{c}")
                    nc.scalar.activation(et[0:tsz], pss[0:tsz, 0:4 * T],
                                         AF.Exp, scale=1.0 / (SCALE * 1024.0))
                    ets.append(et)
                expT.append(ets)
            for cg in range(2):
                ets = expT[cg]
                pot = po.tile([P, 512], f32, tag="po")
                # attnV for both pairs of this chunk-group in one M=128
                # matmul per key-chunk (diagonal blocks valid), then the
                # softmax sums as a second group in the same bank (legal:
                # the attnV group has closed)
                for c, (t0, tsz) in enumerate(TOKC):
                    nc.tensor.matmul(pot[0:P, 0:4 * T],
                                     va[c][0:tsz, 2 * cg:2 * cg + 2, :],
                                     ets[c][0:tsz, :],
                                     start=(c == 0), stop=(c == 1))
                for c, (t0, tsz) in enumerate(TOKC):
                    nc.tensor.matmul(pot[0:1, 256:256 + 4 * T],
                                     ones_col[0:tsz, :], ets[c][0:tsz, :],
                                     start=(c == 0), stop=(c == 1))
                # 1/Z for both pairs at once via exp(-ln Z) on ACT
                lz = apool.tile([1, 4 * T], f32, tag=f"lz{cg}")
                nc.scalar.activation(lz[:], pot[0:1, 256:256 + 4 * T], AF.Ln)
                rrow = apool.tile([1, 4 * T], bf16, tag=f"rr{cg}")
                nc.scalar.activation(rrow[:], lz[:], AF.Exp, scale=-1.0)
                pbb = psc.tile([P, 512], f32, tag="psc")
                nc.tensor.matmul(pbb[0:32, 0:4 * T], ones_row[0:1, 0:32],
                                 rrow[:], start=True, stop=True)
                rb = apool.tile([32, 4 * T], f32, tag=f"rb{cg}")
                if cg == 0:
                    nc.scalar.copy(rb[:], pbb[0:32, 0:4 * T])
                else:
                    nc.vector.tensor_copy(rb[:], pbb[0:32, 0:4 * T])
                for pl in range(2):      # pair-local within the chunk-group
                    for j in range(2):
                        rr0 = 64 * pl + 32 * j
                        cc0 = 98 * pl + 49 * j
                        nc.vector.tensor_tensor(
                            oT8[cg][rr0:rr0 + 32, :],
                            pot[rr0:rr0 + 32, cc0:cc0 + T],
                            rb[0:32, cc0:cc0 + T],
                            op=OP.mult)

            # ---------------- LN2 mean, early: 1024*sum_f rv = ones1024@nv
            #                  + (32*colsum(Wp)) @ (32*oT)
            # LN2 mean, attention part: trails only the oT8 tiles
            for kc in range(2):
                nc.tensor.matmul(shb[0:1, MSUM:MSUM + T], wpcs32[:, kc:kc + 1],
                                 oT8[kc][:], start=(kc == 0), stop=(kc == 1))
            mo2 = apool.tile([1, T], bf16, tag="mo2")
            nc.vector.scalar_tensor_tensor(mo2[:], shb[0:1, MSUM:MSUM + T],
                                           1.0 / (1024.0 * EMB), mr2[0:1, 0],
                                           op0=OP.mult, op1=OP.add)
            nc.tensor.matmul(shb[:, MB2:MB2 + T], ones_row[:], mo2[:],
                             start=True, stop=True)
            # nv_mb = nv - mean2 broadcast, ready before proj lands; then
            # t2 = pp/1024 + nv_mb directly IS rv - mean2 (one STT), and
            # var2 = mean(t2^2) exactly
            nv_mb = apool.tile([P, 2, T], bf16, tag="nv_mb")
            for kc in range(2):
                nc.vector.tensor_tensor(nv_mb[:, kc], nv_bf[:, kc],
                                        shb[:, MB2:MB2 + T], op=OP.subtract)

            # ---------------- projection: plain fp8 (FWL), per-cg accumulate
            pp = pwide.tile([P, 512], f32, tag="pwide")
            for mc in range(2):
                for cg in range(2):
                    nc.tensor.matmul(pp[:, 50 * mc:50 * mc + T],
                                     wpp[:, cg, mc], oT8[cg][:],
                                     start=(cg == 0), stop=(cg == 1))
            pp3 = pp[:, 0:100].rearrange("p (c t) -> p c t", c=2)[:, :, 0:T]
            t2 = apool.tile([P, 2, T], f32, tag="t2")
            nc.vector.scalar_tensor_tensor(t2[:], pp3, 1.0 / 1024.0,
                                           nv_mb[:], op0=OP.mult, op1=OP.add)

            # ---------------- LN2 rstd from t2: rstd = exp(-.5 ln(ssq/N+eps))
            sq2 = apool.tile([P, 2, T], bf16, tag="sq2")
            nc.vector.tensor_tensor(sq2[:], t2[:], t2[:], op=OP.mult)
            for kc in range(2):
                nc.tensor.matmul(shb[0:1, SSQ2:SSQ2 + T], ones_col[:],
                                 sq2[:, kc], start=(kc == 0), stop=(kc == 1))
            rstd_into(mr2[0:1, 1], shb[0:1, SSQ2:SSQ2 + T], '2',
                      scale=1.0 / EMB)
            nc.tensor.matmul(shb[:, MB2 + T:MB2 + 2 * T], ones_row[:],
                             mr2[0:1, 1], start=True, stop=True)
            # lv8 = u2 cast straight to f8 (ln2b is folded into the FFN1
            # bias host-side), so the FFN starts 2 hops earlier; the bf16
            # residual copy + bias ride during the FFN
            lv8 = apool.tile([P, 2, T], f8, tag="lv8")
            for kc in range(2):
                nc.vector.scalar_tensor_tensor(lv8[:, kc], t2[:, kc],
                                               ln2w_c[:, kc:kc + 1],
                                               shb[:, MB2 + T:MB2 + 2 * T],
                                               op0=OP.mult, op1=OP.mult)
            u2bf = apool.tile([P, 2, T], bf16, tag="u2bf")
            for kc in range(2):
                nc.vector.scalar_tensor_tensor(u2bf[:, kc], t2[:, kc],
                                               ln2w_c[:, kc:kc + 1],
                                               shb[:, MB2 + T:MB2 + 2 * T],
                                               op0=OP.mult, op1=OP.mult)
            lvb2 = apool.tile([P, 2, T], bf16, tag="lvb2")
            for kc in range(2):
                nc.vector.tensor_scalar(lvb2[:, kc], u2bf[:, kc],
                                        ln2bb2_c[:, kc:kc + 1], None,
                                        op0=OP.add)
            # gelu-table prefetch, gated on LN2 rstd (last nl_exp-table op)
            gf = AF.Identity if _CACHE.get('sim_ident_gelu') else AF.Gelu
            nc.scalar.activation(dums[0:1, 1:2], mr2[0:1, 1, 0:1], gf)

            # ---------------- FFN fp8 DoubleRow, two-bank psum pipeline
            g8 = []
            for j in range(4):
                g8t = apool.tile([P, 2, T], f8, tag=f"g8_{j}")
                g8.append(g8t)
            for j in range(4):
                ph = psc.tile([P, 512], f32, tag="psc")
                for jj in range(2):
                    mc = 2 * j + jj
                    nc.tensor.matmul(ph[:, 50 * jj:50 * jj + T], w1[:, mc],
                                     lv8[:], start=True, stop=True,
                                     perf_mode=DR)
                    nc.scalar.activation(g8[j][:, jj],
                                         ph[:, 50 * jj:50 * jj + T],
                                         gf, scale=1.0 / WS,
                                         bias=b1_c[:, mc:mc + 1])
            for mc in range(2):
                pf = pwide.tile([P, 512], f32, tag="pwide")
                for i in range(4):
                    nc.tensor.matmul(pf[:, 0:T], w2[:, mc, i], g8[i][:],
                                     start=(i == 0), stop=(i == 3),
                                     perf_mode=DR)
                ot = apool.tile([P, T], f32, tag=f"out{mc}")
                nc.vector.scalar_tensor_tensor(ot[:], pf[:, 0:T],
                                               1.0 / WS, lvb2[:, mc],
                                               op0=OP.mult, op1=OP.add)
                # both output chunks on HWDGE queues (sync + scalar): the
                # SWDGE path measured ~1us slower transfer setup on inputs
                eng = nc.sync if mc == 0 else nc.scalar
                eng.dma_start(out_d[:, mc * T:(mc + 1) * T], ot[:],
                              single_packet=True)

    nc.compile()
    return nc


# ---------------------------------------------------------------- host side
def _reorder_qkv(W, b):
    W4 = np.asarray(W, np.float32).reshape(EMB, H, DH, 3)
    b4 = np.asarray(b, np.float32).reshape(H, DH, 3)
    return ([np.ascontiguousarray(W4[:, :, :, i].reshape(EMB, EMB)) for i in range(3)],
            [np.ascontiguousarray(b4[:, :, i].reshape(EMB)) for i in range(3)])


def _pack_w8(w):
    """(K, M) f32 -> x32-scaled partition-major (128, K//128 * M) f8 block."""
    w = np.asarray(w, np.float32) * WS
    k, m = w.shape
    c = k // P
    return np.transpose(w.reshape(c, P, m), (1, 0, 2)).reshape(P, c * m).astype(F8)


def _ileave(pair3):
    """[P, 2, m] -> [P, 2m]: A/B column-interleaved, columns reversed
    (DoubleRowSwInterleave weight layout)."""
    blk = pair3[:, :, ::-1]
    return np.ascontiguousarray(np.transpose(blk, (0, 2, 1)).reshape(P, -1))


def _pack_w8i(w, mc_cols, mc_major=False):
    """(K, M) f32 -> x32-scaled SwInterleave layout: one [P, 2*mc_cols]
    interleaved block per (k-tile pair, M-chunk)."""
    w = np.asarray(w, np.float32) * WS
    k, m = w.shape
    c = k // P
    w3 = np.transpose(w.reshape(c, P, m), (1, 0, 2)).astype(F8)  # [P, c, m]
    blk = [[_ileave(w3[:, 2 * i:2 * i + 2, j0:j0 + mc_cols])
            for j0 in range(0, m, mc_cols)] for i in range(c // 2)]
    if mc_major:
        out = [blk[i][j] for j in range(len(blk[0])) for i in range(len(blk))]
    else:
        out = [blk[i][j] for i in range(len(blk)) for j in range(len(blk[0]))]
    return np.concatenate(out, axis=1)


def _pack_x(x):
    """(tokens, 256) -> (128, 2*tokens) f32 partition-major transposed."""
    xt = np.ascontiguousarray(np.asarray(x, np.float32).T)       # (256, t)
    t = xt.shape[1]
    return np.transpose(xt.reshape(2, P, t), (1, 0, 2)).reshape(P, 2 * t)


def _f8cols(a):
    """(128, n) f8 array -> zero-padded f32-col view (n_f32 = ceil(n/4))."""
    n = a.shape[1]
    pad = (-n) % 4
    if pad:
        a = np.concatenate([a, np.zeros((P, pad), F8)], axis=1)
    return np.ascontiguousarray(a).view(np.float32)


def _bfcols(a):
    """(128, n) bf16 array -> zero-padded f32-col view."""
    n = a.shape[1]
    pad = (-n) % 2
    if pad:
        a = np.concatenate([a, np.zeros((P, pad), BF)], axis=1)
    return np.ascontiguousarray(a).view(np.float32)


def _cols(v):
    """(256,) -> (128, 2) natural feature chunks."""
    return np.ascontiguousarray(np.asarray(v, np.float32).reshape(2, P).T)


def make_in_maps(inputs):
    inp = {k: np.asarray(v, np.float32) for k, v in inputs.items()}
    qkv_v = _reorder_qkv(inp['Wqkv_v'], inp['bqkv_v'])
    qkv_i = _reorder_qkv(inp['Wqkv_i'], inp['bqkv_i'])
    out = []
    for core in range(NCORES):
        r0 = (core % 4) * T
        if core // 4 == 0:  # vis output: vis queries, ir keys/values
            x_own, x_oth = inp['vis_emb'][0], inp['ir_emb'][0]
            wq, bq = qkv_v[0][0], qkv_v[1][0]
            wk = qkv_i[0][1]
            wv, bv = qkv_i[0][2], qkv_i[1][2]
            wp, bp = inp['Wp_v'], inp['bp_v']
            lnp = (inp['ln1v_w'], inp['ln1v_b'], inp['ln2v_w'], inp['ln2v_b'])
            w1, b1, w2, b2 = inp['W1v'], inp['b1v'], inp['W2v'], inp['b2v']
        else:               # ir output: ir queries, vis kv
            x_own, x_oth = inp['ir_emb'][0], inp['vis_emb'][0]
            wq, bq = qkv_i[0][0], qkv_i[1][0]
            wk = qkv_v[0][1]
            wv, bv = qkv_v[0][2], qkv_v[1][2]
            wp, bp = inp['Wp_i'], inp['bp_i']
            lnp = (inp['ln1i_w'], inp['ln1i_b'], inp['ln2i_w'], inp['ln2i_b'])
            w1, b1, w2, b2 = inp['W1i'], inp['b1i'], inp['W2i'], inp['b2i']

        # fold v-bias into the projection bias: softmax rows sum to 1
        bp_f = np.asarray(bp, np.float32) + np.asarray(bv, np.float32) @ np.asarray(wp, np.float32)

        # ln2-bias folded into the FFN1 bias: gelu((u2+ln2b)@W1 + b1)
        # = gelu(u2@W1 + (ln2b@W1 + b1))
        b1_f = (np.asarray(b1, np.float32)
                + np.asarray(lnp[3], np.float32) @ np.asarray(w1, np.float32))
        aux = np.zeros((P, 32), np.float32)
        aux[:, 0:2] = _cols(np.asarray(bq, np.float32) * WS)
        aux[:, 2:10] = b1_f.reshape(8, P).T
        aux[:, 10:12] = _cols(np.asarray(lnp[1], np.float32) + bp_f)
        aux[:, 12:14] = _cols(lnp[3])
        aux[:, 14:16] = _cols(lnp[0])
        aux[:, 16:18] = _cols(lnp[2])
        aux[:, 18:20] = _cols(np.asarray(lnp[3], np.float32)
                              + np.asarray(b2, np.float32))
        wpcs = np.zeros((P, 4), F8)
        wpcs[:, 0:2] = (np.asarray(wp, np.float32).sum(axis=1) * WS
                        ).reshape(2, P).T.astype(F8)
        aux[:, 20] = wpcs.view(np.float32)[:, 0]

        xq_f = _pack_x(x_own[r0:r0 + T])            # [128, 98] f32
        xkv_f = _pack_x(x_oth)                      # [128, 392] f32
        xkv3 = np.zeros((P, 2, 256), F8)
        xkv3[:, :, 0:N] = xkv_f.astype(F8).reshape(P, 2, N)
        xkvi = np.concatenate(
            [_ileave(xkv3[:, :, 0:128]), _ileave(xkv3[:, :, 128:256])], axis=1)

        blobQ = np.concatenate([
            _f8cols(_pack_w8i(wq, P)),                  # 128
            _f8cols(xq_f.astype(F8)),                   # 25
        ], axis=1)
        blobK = np.concatenate([
            _f8cols(_pack_w8i(wk, P)),                  # 128
            _f8cols(xkv_f.astype(F8)),                  # 98
        ], axis=1)
        blobA = np.concatenate([
            aux,                                        # 32
            _bfcols(xq_f.astype(BF)),                   # 49
        ], axis=1)
        blobV = np.concatenate([
            _f8cols(_pack_w8(wv)),                      # 128
            _f8cols(xkvi),                              # 128
        ], axis=1)
        blobP = _f8cols(_pack_w8(wp))                   # 128
        blobF = np.concatenate([
            _f8cols(_pack_w8i(w1, P)),                  # 512
            _f8cols(_pack_w8i(w2, P, mc_major=True)),   # 512
        ], axis=1)
        out.append({
            'blobQ': np.ascontiguousarray(blobQ),
            'blobK': np.ascontiguousarray(blobK),
            'blobA': np.ascontiguousarray(blobA),
            'blobV': np.ascontiguousarray(blobV),
            'blobP': np.ascontiguousarray(blobP),
            'blobF': np.ascontiguousarray(blobF),
        })
    return out


def _recon(x):
    x = x.reshape(14, 14, 16, 16)
    x = np.transpose(x, (2, 3, 0, 1))
    return x.reshape(1, 1, 224, 224)


def assemble(core_outs):
    # core out [128, 98] f32 -> [49 tokens, 256 feats]
    toks = [np.transpose(np.asarray(o, np.float32).reshape(P, 2, T),
                         (2, 1, 0)).reshape(T, EMB) for o in core_outs]
    ov = np.concatenate(toks[0:4], axis=0)
    oi = np.concatenate(toks[4:8], axis=0)
    return np.concatenate([_recon(oi), _recon(ov)], axis=1).astype(np.float32)


def get_nc():
    if 'nc' not in _CACHE:
        _CACHE['nc'] = build_bass()
    return _CACHE['nc']


def kernel(**inputs):
    from concourse import bass_utils
    nc = get_nc()
    in_maps = make_in_maps(inputs)
    res = bass_utils.run_bass_kernel_spmd(nc, in_maps, core_ids=list(range(NCORES)))
    outs = [np.asarray(r['out'], np.float32) for r in res.results]
    return assemble(outs)
